# revision 1
# baseline (speedup 1.0000x reference)
"""Cross-attention (RMSNorm + QKV proj + 2D RoPE + SDPA + out-proj) on 8
Trainium2 NeuronCores.

Sharding: 8 cores = 4 batches x 2 query-halves. Each core computes the full
KV projection for its batch (duplicated across the 2 cores sharing a batch)
and attention + output projection for its 512 query rows. No collectives.

On-device layout is feature-major ("transposed"): activations live as
[feature, seq] with features on SBUF partitions. Host pre-transposes inputs
and weights so every linear layer is a plain lhsT.T @ rhs PE matmul whose
output is again feature-major. Head dims are de-interleaved (even rot dims
then odd rot dims per head) so RoPE's pair rotation becomes a 32-partition
block swap (SBUF->SBUF DMA) plus elementwise DVE math, with the sin sign
folded into host-negated frequency rows.

Matmuls run in float32r mode (full PE rate, ~1.5e-4 rel err); the RoPE angle
matmuls stay fp32 exact. Softmax skips max-subtraction (logits are O(1) by
construction) and the denominator comes free as a 65th ones-column in the
AV matmul; normalization happens before the output projection.
"""

import numpy as np

B, SQ, SK, D = 4, 1024, 1024, 768
H, HD = 12, 64
DC = D // 128          # 6 feature chunks
SQL = SQ // 2          # 512 query rows per core
SKC = SK // 128        # 8 key chunks
EPS = 1e-5
PI = float(np.pi)
TWOPI = 2.0 * PI
INV2PI = 1.0 / TWOPI
RBIG = 12582912.0      # 1.5 * 2**23: fp32 round-to-nearest-integer trick
NCORES = 8

_cache = {}


# ---------------------------------------------------------------------------
# compiler workarounds
# ---------------------------------------------------------------------------

def _apply_patches():
    """This walrus build allows only ONE sync-wait command per instruction.
    (a) split the Tile kernel-tail drain into one drain per waited proc;
    (b) post-process the BIR JSON, moving excess waits onto same-engine NoOps
    inserted immediately before the over-subscribed instruction."""
    import json
    import concourse.tile as tile
    import concourse.bass as cbass
    from concourse.vector_clock import ScopedClock, VectorClock

    if getattr(cbass.Bass, "_wait_split_patched", False):
        return

    def _drain_and_barrier(self, tick_clock, wait_clock):
        gc = tick_clock.global_clock
        try:
            vec = gc[None]
        except Exception:
            vec = gc
        n = len(vec)
        for p in [i for i in range(n) if vec[i] > 0]:
            sub = [0] * n
            sub[p] = vec[p]
            inst = self.nc.sync.drain()
            wait_clock.add_sem_waits(inst.ins, ScopedClock({None: VectorClock(sub)}))
        self.nc.all_engine_barrier()
        assert self.sems is not None
        popped = self.nc._tile_sem_poison_stack.pop()
        assert popped is self._sem_poison
        self.nc.clear_and_free_semaphores(list(self.sems.allocated().values()))
        self.nc.all_engine_barrier()

    tile.TileContext._drain_and_barrier = _drain_and_barrier

    def _split_waits(bir):
        for f in bir.get("functions", []):
            for blk in f.get("blocks", []):
                insts = blk.get("instructions")
                if not insts:
                    continue
                out = []
                ctr = 0
                for inst in insts:
                    si = inst.get("sync_info")
                    ow = (si or {}).get("on_wait") or []
                    if len(ow) > 1:
                        for w in ow[:-1]:
                            nop = {
                                "name": f"{inst['name']}-ws{ctr}",
                                "opcode": "NoOp",
                                "engine": inst.get("engine"),
                                "ins": [],
                                "outs": [],
                                "sync_info": {"on_wait": [w], "on_update": []},
                            }
                            if "debug" in inst:
                                nop["debug"] = inst["debug"]
                            ctr += 1
                            out.append(nop)
                        si["on_wait"] = [ow[-1]]
                    out.append(inst)
                blk["instructions"] = out
        return bir

    orig = cbass.Bass.to_json_bytes

    def to_json_bytes(self, *a, **kw):
        return json.dumps(_split_waits(json.loads(orig(self, *a, **kw)))).encode()

    cbass.Bass.to_json_bytes = to_json_bytes
    cbass.Bass._wait_split_patched = True


# ---------------------------------------------------------------------------
# device program
# ---------------------------------------------------------------------------

def _build_nc():
    import concourse.bass as bass
    import concourse.tile as tile
    import concourse.mybir as mybir

    F32 = mybir.dt.float32
    F32R = mybir.dt.float32r
    AF = mybir.ActivationFunctionType
    ALU = mybir.AluOpType

    nc = bass.Bass()

    qT_d = nc.dram_tensor("qT", [D, SQL], F32R, kind="ExternalInput")
    kvT_d = nc.dram_tensor("kvT", [D, SK], F32R, kind="ExternalInput")
    posq_d = nc.dram_tensor("posqT", [2, SQL], F32, kind="ExternalInput")
    posk_d = nc.dram_tensor("poskT", [2, SK], F32, kind="ExternalInput")
    freqs_d = nc.dram_tensor("freqsT", [2, D], F32, kind="ExternalInput")
    wq_d = nc.dram_tensor("wqT", [D, D], F32R, kind="ExternalInput")
    wk_d = nc.dram_tensor("wkT", [D, D], F32R, kind="ExternalInput")
    wv_d = nc.dram_tensor("wvT", [D, D], F32R, kind="ExternalInput")
    wo_d = nc.dram_tensor("woT", [D, D], F32R, kind="ExternalInput")
    bq_d = nc.dram_tensor("bqR", [128, DC], F32, kind="ExternalInput")
    bk_d = nc.dram_tensor("bkR", [128, DC], F32, kind="ExternalInput")
    bo_d = nc.dram_tensor("boR", [128, DC], F32, kind="ExternalInput")
    ones_d = nc.dram_tensor("ones128", [128, 128], F32R, kind="ExternalInput")
    onesc_d = nc.dram_tensor("onescol", [128, H], F32R, kind="ExternalInput")
    out_d = nc.dram_tensor("outT", [D, SQL], F32, kind="ExternalOutput")

    den_d = nc.dram_tensor("den_scratch", [H, SQL], F32, kind="Internal")

    with tile.TileContext(nc) as tc:
        import contextlib
        ctx = contextlib.ExitStack()
        with ctx:
            persist = ctx.enter_context(tc.tile_pool(name="persist", bufs=1))
            tmp = ctx.enter_context(tc.tile_pool(name="tmp", bufs=2))
            ps = ctx.enter_context(tc.tile_pool(name="ps", bufs=6, space="PSUM"))
            pso = ctx.enter_context(tc.tile_pool(name="pso", bufs=2, space="PSUM"))

            # ---- persistent small tensors -------------------------------
            freqs_sb = persist.tile([2, D], F32)
            posq_sb = persist.tile([2, SQL], F32)
            posk_sb = persist.tile([2, SK], F32)
            bq_sb = persist.tile([128, DC], F32)
            bk_sb = persist.tile([128, DC], F32)
            bo_sb = persist.tile([128, DC], F32)
            ones_sb = persist.tile([128, 128], F32R)
            onesc_sb = persist.tile([128, H], F32R)
            halfpi = persist.tile([128, 1], F32)
            eps_t = persist.tile([128, 1], F32)
            den12 = persist.tile([H, SQL], F32)
            nc.sync.dma_start(out=freqs_sb, in_=freqs_d[:, :])
            nc.sync.dma_start(out=posq_sb, in_=posq_d[:, :])
            nc.sync.dma_start(out=posk_sb, in_=posk_d[:, :])
            nc.sync.dma_start(out=bq_sb, in_=bq_d[:, :])
            nc.sync.dma_start(out=bk_sb, in_=bk_d[:, :])
            nc.sync.dma_start(out=bo_sb, in_=bo_d[:, :])
            nc.sync.dma_start(out=ones_sb, in_=ones_d[:, :])
            nc.sync.dma_start(out=onesc_sb, in_=onesc_d[:, :])
            nc.vector.memset(halfpi, PI / 2)
            nc.vector.memset(eps_t, EPS)

            # ---- persistent activations ---------------------------------
            qrot = [persist.tile([128, SQL], F32R, name=f"qrot{c}") for c in range(DC)]
            krot = [persist.tile([128, SK], F32R, name=f"krot{c}") for c in range(DC)]
            vp = [persist.tile([128, H, HD + 1], F32R, name=f"vp{c}") for c in range(SKC)]
            oT = [persist.tile([128, SQL], F32R, name=f"oT{c}") for c in range(DC)]

            # ---- phase-1 inputs -----------------------------------------
            ph1 = ctx.enter_context(tc.tile_pool(name="ph1", bufs=1))
            qT = [ph1.tile([128, SQL], F32R, name=f"qT{c}") for c in range(DC)]
            kvT = [ph1.tile([128, SK], F32R, name=f"kvT{c}") for c in range(DC)]
            for c in range(DC):
                nc.sync.dma_start(out=qT[c], in_=qT_d[c * 128:(c + 1) * 128, :])
                nc.sync.dma_start(out=kvT[c], in_=kvT_d[c * 128:(c + 1) * 128, :])

            def wchunk(dram, c):
                t = tmp.tile([128, D], F32R, tag="wchunk", name="wch")
                nc.sync.dma_start(out=t, in_=dram[c * 128:(c + 1) * 128, :])
                return t

            # ---- RMSNorm over features (partition dim) ------------------
            ss = ps.tile([128, SQL], F32, tag="p512")
            for c in range(DC):
                sq = tmp.tile([128, SQL], F32R, tag="e512", bufs=4, name="sq")
                nc.vector.tensor_mul(out=sq, in0=qT[c], in1=qT[c])
                nc.tensor.matmul(ss, ones_sb, sq, start=(c == 0), stop=(c == DC - 1))
            sq_t = tmp.tile([128, SQL], F32, tag="outc")
            nc.scalar.activation(out=sq_t, in_=ss, func=AF.Sqrt, bias=eps_t,
                                 scale=1.0 / D)
            rstd = tmp.tile([128, SQL], F32, tag="outc", name="rstd")
            nc.vector.reciprocal(out=rstd, in_=sq_t)
            for c in range(DC):
                nc.vector.tensor_mul(out=qT[c], in0=qT[c], in1=rstd)

            # ---- helpers ------------------------------------------------
            def rope_reduce(ps_ang, width, sin_dst, cos_dst):
                """sin/cos of angles in ps_ang [128,width], with fp32
                round-to-nearest range reduction into [-pi, pi]."""
                t2 = tmp.tile([128, 512], F32, tag="rr", bufs=3, name="t2")[:, :width]
                nc.vector.tensor_scalar(out=t2, in0=ps_ang, scalar1=INV2PI,
                                        scalar2=RBIG, op0=ALU.mult, op1=ALU.add)
                kk = tmp.tile([128, 512], F32, tag="rr", bufs=3, name="kk")[:, :width]
                nc.vector.tensor_scalar_add(out=kk, in0=t2, scalar1=-RBIG)
                red = tmp.tile([128, 512], F32, tag="rr", bufs=3, name="red")[:, :width]
                nc.vector.scalar_tensor_tensor(out=red, in0=kk, scalar=-TWOPI,
                                               in1=ps_ang, op0=ALU.mult,
                                               op1=ALU.add)
                nc.scalar.activation(out=sin_dst, in_=red, func=AF.Sin)
                t2c = tmp.tile([128, 512], F32, tag="rr", bufs=3, name="t2c")[:, :width]
                nc.vector.tensor_scalar(out=t2c, in0=ps_ang, scalar1=INV2PI,
                                        scalar2=RBIG + 0.25, op0=ALU.mult,
                                        op1=ALU.add)
                kkc = tmp.tile([128, 512], F32, tag="rr", bufs=3, name="kkc")[:, :width]
                nc.vector.tensor_scalar_add(out=kkc, in0=t2c, scalar1=-RBIG)
                redc = tmp.tile([128, 512], F32, tag="rr", bufs=3, name="redc")[:, :width]
                nc.vector.scalar_tensor_tensor(out=redc, in0=kkc, scalar=-TWOPI,
                                               in1=ps_ang, op0=ALU.mult,
                                               op1=ALU.add)
                nc.scalar.activation(out=cos_dst, in_=redc, func=AF.Sin,
                                     bias=halfpi)

            def block_swap(dst, src, width):
                for base in (0, 64):
                    nc.gpsimd.dma_start(out=dst[base:base + 32, :width],
                                        in_=src[base + 32:base + 64, :width])
                    nc.gpsimd.dma_start(out=dst[base + 32:base + 64, :width],
                                        in_=src[base:base + 32, :width])

            # ---- Q projection (c-outer, 6 psum accumulators) ------------
            pq6 = [ps.tile([128, 512], F32, tag="p512", name=f"pq{m}")
                   for m in range(DC)]
            for c in range(DC):
                wc = wchunk(wq_d, c)
                for m in range(DC):
                    nc.tensor.matmul(pq6[m], wc[:, m * 128:(m + 1) * 128], qT[c],
                                     start=(c == 0), stop=(c == DC - 1))
            # ---- RoPE on Q (bias copy interleaved per chunk) ------------
            for m in range(DC):
                qp = tmp.tile([128, 512], F32, tag="pre", bufs=3, name="qp")
                nc.scalar.activation(out=qp, in_=pq6[m], func=AF.Identity,
                                     bias=bq_sb[:, m:m + 1])
                pa = ps.tile([128, 512], F32, tag="p512", name="pa")
                nc.tensor.matmul(pa, freqs_sb[:, m * 128:(m + 1) * 128], posq_sb,
                                 start=True, stop=True)
                sin_t = tmp.tile([128, 512], F32, tag="sin", name="sin_t")
                cos_t = tmp.tile([128, 512], F32, tag="cos", name="cos_t")
                rope_reduce(pa, SQL, sin_t, cos_t)
                t1 = tmp.tile([128, 512], F32, tag="cmb", name="t1")
                block_swap(t1, qp, SQL)
                nc.vector.tensor_mul(out=t1, in0=t1, in1=sin_t)
                nc.vector.tensor_mul(out=qrot[m], in0=qp, in1=cos_t)
                nc.vector.tensor_add(out=qrot[m], in0=qrot[m], in1=t1)

            # ---- K projection + RoPE (per 512-half, c-outer) ------------
            for half in range(2):
                hs = slice(half * 512, half * 512 + 512)
                pk6 = [ps.tile([128, 512], F32, tag="p512", name=f"pk{m}")
                       for m in range(DC)]
                for c in range(DC):
                    wc = wchunk(wk_d, c)
                    for m in range(DC):
                        nc.tensor.matmul(pk6[m], wc[:, m * 128:(m + 1) * 128],
                                         kvT[c][:, hs],
                                         start=(c == 0), stop=(c == DC - 1))
                for m in range(DC):
                    kp = tmp.tile([128, 512], F32, tag="pre", bufs=3, name="kp")
                    nc.scalar.activation(out=kp, in_=pk6[m],
                                         func=AF.Identity, bias=bk_sb[:, m:m + 1])
                    pa = ps.tile([128, 512], F32, tag="p512", name="pa")
                    nc.tensor.matmul(pa, freqs_sb[:, m * 128:(m + 1) * 128],
                                     posk_sb[:, hs], start=True, stop=True)
                    sin_t = tmp.tile([128, 512], F32, tag="sin", name="sin_t")
                    cos_t = tmp.tile([128, 512], F32, tag="cos", name="cos_t")
                    rope_reduce(pa, 512, sin_t, cos_t)
                    t1 = tmp.tile([128, 512], F32, tag="cmb", name="t1")
                    block_swap(t1, kp, 512)
                    nc.vector.tensor_mul(out=t1, in0=t1, in1=sin_t)
                    nc.vector.tensor_mul(out=krot[m][:, hs], in0=kp, in1=cos_t)
                    nc.vector.tensor_add(out=krot[m][:, hs], in0=krot[m][:, hs],
                                         in1=t1)

            # ---- V projection (row-major, ones column appended) ---------
            wv = []
            for c in range(DC):
                t = ph1.tile([128, D], F32R, tag=f"wv{c}")
                nc.sync.dma_start(out=t, in_=wv_d[c * 128:(c + 1) * 128, :])
                wv.append(t)
            for kc in range(SKC):
                ksl = slice(kc * 128, (kc + 1) * 128)
                pv0 = ps.tile([128, 512], F32, tag="p512")
                pv1 = ps.tile([128, 512], F32, tag="p512")
                for c in range(DC):
                    nc.tensor.matmul(pv0, kvT[c][:, ksl], wv[c][:, 0:512],
                                     start=(c == 0), stop=(c == DC - 1))
                    nc.tensor.matmul(pv1[:, 0:256], kvT[c][:, ksl],
                                     wv[c][:, 512:768],
                                     start=(c == 0), stop=(c == DC - 1))
                nc.vector.tensor_copy(
                    out=vp[kc][:, 0:8, 0:HD],
                    in_=pv0.rearrange("p (h d) -> p h d", h=8))
                nc.vector.tensor_copy(
                    out=vp[kc][:, 8:12, 0:HD],
                    in_=pv1[:, 0:256].rearrange("p (h d) -> p h d", h=4))
                nc.gpsimd.dma_start(out=vp[kc][:, :, HD], in_=onesc_sb)

            # ---- attention ----------------------------------------------
            for h in range(H):
                mh, off = h // 2, 64 * (h % 2)
                po = pso.tile([65, 512], F32, tag="po", name="po")
                for kc in range(SKC):
                    pss = ps.tile([128, 512], F32, tag="p512", name="pss")
                    nc.tensor.matmul(
                        pss,
                        krot[mh][off:off + 64, kc * 128:(kc + 1) * 128],
                        qrot[mh][off:off + 64, :],
                        start=True, stop=True)
                    ex = tmp.tile([128, SQL], F32R, tag="e512", bufs=4, name="ex")
                    nc.scalar.activation(out=ex, in_=pss, func=AF.Exp,
                                         scale=1.0 / 8.0)
                    nc.tensor.matmul(po, vp[kc][:, h, :], ex,
                                     start=(kc == 0), stop=(kc == SKC - 1))
                # stash unnormalized O and the denominator row
                nc.vector.tensor_copy(out=oT[mh][off:off + 64, :], in_=po[0:64, :])
                drow = tmp.tile([1, SQL], F32, tag="drow", name="drow")
                nc.vector.tensor_copy(out=drow, in_=po[64:65, :])
                nc.sync.dma_start(out=den12[h:h + 1, :], in_=drow)

            # normalize: batched reciprocal + partition-broadcast via DRAM
            nc.vector.reciprocal(out=den12, in_=den12)
            nc.sync.dma_start(out=den_d[:, :], in_=den12)
            import concourse.bass as bass_mod
            for mh in range(DC):
                rb2 = tmp.tile([128, SQL], F32, tag="outc", name="rb2")
                for j in range(2):
                    row = den_d[2 * mh + j, :]
                    bsrc = bass_mod.AP(tensor=row.tensor, offset=row.offset,
                                       ap=[[0, 64], *row.ap])
                    nc.sync.dma_start(out=rb2[64 * j:64 * j + 64, :], in_=bsrc)
                nc.vector.tensor_mul(out=oT[mh], in0=oT[mh], in1=rb2)

            # ---- output projection (c-outer, 6 psum accumulators) -------
            po6 = [ps.tile([128, 512], F32, tag="p512", name=f"po6_{m}")
                   for m in range(DC)]
            for c in range(DC):
                wc = wchunk(wo_d, c)
                for m in range(DC):
                    nc.tensor.matmul(po6[m], wc[:, m * 128:(m + 1) * 128], oT[c],
                                     start=(c == 0), stop=(c == DC - 1))
            for m in range(DC):
                outc = tmp.tile([128, SQL], F32, tag="outc", name="outc")
                nc.scalar.activation(out=outc, in_=po6[m], func=AF.Identity,
                                     bias=bo_sb[:, m:m + 1])
                nc.sync.dma_start(out=out_d[m * 128:(m + 1) * 128, :], in_=outc)

    return nc


# ---------------------------------------------------------------------------
# host wrapper
# ---------------------------------------------------------------------------

def kernel(q, kv, posq, posk, w_norm, w_q, b_q, w_kv, b_kv, w_out, b_out, freqs):
    _apply_patches()
    from concourse.bass_utils import run_bass_kernel_spmd

    q = np.asarray(q, np.float32)
    kv = np.asarray(kv, np.float32)
    posq_np = np.asarray(posq)
    posk_np = np.asarray(posk)
    w_norm = np.asarray(w_norm, np.float32)
    w_q = np.asarray(w_q, np.float32)
    b_q = np.asarray(b_q, np.float32)
    w_kv = np.asarray(w_kv, np.float32)
    b_kv = np.asarray(b_kv, np.float32)
    w_out = np.asarray(w_out, np.float32)
    b_out = np.asarray(b_out, np.float32)
    freqs = np.asarray(freqs, np.float32)

    # de-interleave head dims: new j<32 -> old 2j (even), j>=32 -> old 2(j-32)+1
    perm = np.empty(D, np.int64)
    for h in range(H):
        for j in range(HD):
            perm[h * HD + j] = h * HD + (2 * j if j < 32 else 2 * (j - 32) + 1)

    wqT = np.ascontiguousarray((w_q[perm, :] * w_norm[None, :]).T)
    wkT = np.ascontiguousarray(w_kv[:D][perm, :].T)
    wvT = np.ascontiguousarray(w_kv[D:].T)
    woT = np.ascontiguousarray(w_out.T)
    bqR = np.ascontiguousarray(b_q[perm].reshape(DC, 128).T)
    bkR = np.ascontiguousarray(b_kv[:D][perm].reshape(DC, 128).T)
    bo_eff = b_out + w_out @ b_kv[D:]          # fold V bias (softmax sums to 1)
    boR = np.ascontiguousarray(bo_eff.reshape(DC, 128).T)

    # frequency rows in de-interleaved layout; e-rows negated so that
    # sin(ang_signed) carries the rotation sign
    fr = np.empty((2, D), np.float32)
    for h in range(H):
        f = freqs[:, h, :]                      # [2, 32]
        fr[:, h * HD:h * HD + 32] = -f
        fr[:, h * HD + 32:(h + 1) * HD] = f

    ones128 = np.ones((128, 128), np.float32)
    onescol = np.ones((128, H), np.float32)

    if "nc" not in _cache:
        _cache["nc"] = _build_nc()
    nc = _cache["nc"]

    in_maps = []
    for core in range(NCORES):
        b, half = core // 2, core % 2
        qs = slice(half * SQL, (half + 1) * SQL)
        in_maps.append({
            "qT": np.ascontiguousarray(q[b, qs, :].T),
            "kvT": np.ascontiguousarray(kv[b].T),
            "posqT": np.ascontiguousarray(posq_np[b, qs, :].T.astype(np.float32)),
            "poskT": np.ascontiguousarray(posk_np[b].T.astype(np.float32)),
            "freqsT": fr,
            "wqT": wqT, "wkT": wkT, "wvT": wvT, "woT": woT,
            "bqR": bqR, "bkR": bkR, "boR": boR,
            "ones128": ones128, "onescol": onescol,
        })

    res = run_bass_kernel_spmd(nc, in_maps, core_ids=list(range(NCORES)))
    kernel._last_result = res

    out = np.empty((B, SQ, D), np.float32)
    for core in range(NCORES):
        b, half = core // 2, core % 2
        out[b, half * SQL:(half + 1) * SQL, :] = res.results[core]["outT"].T
    return out



# revision 3
# speedup vs baseline: 1.1749x; 1.1749x over previous
"""Cross-attention (RMSNorm + QKV proj + 2D RoPE + SDPA + out-proj) on 8
Trainium2 NeuronCores.

Sharding: 8 cores = 4 batches x 2 query-halves. Each core computes the full
KV projection for its batch (duplicated across the 2 cores sharing a batch)
and attention + output projection for its 512 query rows. No collectives.

On-device layout is feature-major: activations live as [feature, seq] with
features on SBUF partitions. Host pre-transposes inputs and weights (fp16)
so every linear layer is a plain lhsT.T @ rhs PE matmul at full rate. Head
dims are de-interleaved (even rot dims then odd rot dims per head) so RoPE's
pair rotation becomes a 32-partition block swap plus two multiplies against
host-precomputed sin/cos tables (fp16, sign folded into the sin rows).

RMSNorm's rsqrt is computed as exp(-0.5*ln(x)) so the whole kernel needs a
single ACT table set (natural_log_exp). Softmax skips max-subtraction
(logits are O(1) by construction); the denominator comes free as a 65th
ones-column in the AV matmul. Scores for three key chunks accumulate into a
3-bank PSUM tile so each softmax exp covers N=1536 at once. The per-head
denominator reciprocal + broadcast and the per-head-pair output projection
(accumulated in SBUF) pipeline behind later heads' attention.
"""

import numpy as np

B, SQ, SK, D = 4, 1024, 1024, 768
H, HD = 12, 64
DC = D // 128          # 6 feature chunks
SQL = SQ // 2          # 512 query rows per core
SKC = SK // 128        # 8 key chunks
EPS = 1e-5
NCORES = 8

_cache = {}


# ---------------------------------------------------------------------------
# compiler workarounds
# ---------------------------------------------------------------------------

def _apply_patches():
    """This walrus build allows only ONE sync-wait command per instruction.
    (a) split the Tile kernel-tail drain into one drain per waited proc;
    (b) post-process the BIR JSON, moving excess waits onto same-engine NoOps
    inserted immediately before the over-subscribed instruction."""
    import json
    import concourse.tile as tile
    import concourse.bass as cbass
    from concourse.vector_clock import ScopedClock, VectorClock

    if getattr(cbass.Bass, "_wait_split_patched", False):
        return

    def _drain_and_barrier(self, tick_clock, wait_clock):
        gc = tick_clock.global_clock
        try:
            vec = gc[None]
        except Exception:
            vec = gc
        n = len(vec)
        for p in [i for i in range(n) if vec[i] > 0]:
            sub = [0] * n
            sub[p] = vec[p]
            inst = self.nc.sync.drain()
            wait_clock.add_sem_waits(inst.ins, ScopedClock({None: VectorClock(sub)}))
        self.nc.all_engine_barrier()
        assert self.sems is not None
        popped = self.nc._tile_sem_poison_stack.pop()
        assert popped is self._sem_poison
        self.nc.clear_and_free_semaphores(list(self.sems.allocated().values()))
        self.nc.all_engine_barrier()

    tile.TileContext._drain_and_barrier = _drain_and_barrier

    def _split_waits(bir):
        for f in bir.get("functions", []):
            for blk in f.get("blocks", []):
                insts = blk.get("instructions")
                if not insts:
                    continue
                out = []
                ctr = 0
                for inst in insts:
                    si = inst.get("sync_info")
                    ow = (si or {}).get("on_wait") or []
                    if len(ow) > 1:
                        for w in ow[:-1]:
                            nop = {
                                "name": f"{inst['name']}-ws{ctr}",
                                "opcode": "NoOp",
                                "engine": inst.get("engine"),
                                "ins": [],
                                "outs": [],
                                "sync_info": {"on_wait": [w], "on_update": []},
                            }
                            if "debug" in inst:
                                nop["debug"] = inst["debug"]
                            ctr += 1
                            out.append(nop)
                        si["on_wait"] = [ow[-1]]
                    out.append(inst)
                blk["instructions"] = out
        return bir

    orig = cbass.Bass.to_json_bytes

    def to_json_bytes(self, *a, **kw):
        return json.dumps(_split_waits(json.loads(orig(self, *a, **kw)))).encode()

    cbass.Bass.to_json_bytes = to_json_bytes
    cbass.Bass._wait_split_patched = True


# ---------------------------------------------------------------------------
# device program
# ---------------------------------------------------------------------------

def _build_nc():
    import concourse.bass as bass
    import concourse.tile as tile
    import concourse.mybir as mybir

    F32 = mybir.dt.float32
    F32R = mybir.dt.float32r
    F16 = mybir.dt.float16
    AF = mybir.ActivationFunctionType
    ALU = mybir.AluOpType

    nc = bass.Bass()

    qT_d = nc.dram_tensor("qT", [D, SQL], F16, kind="ExternalInput")
    kvT_d = nc.dram_tensor("kvT", [D, SK], F16, kind="ExternalInput")
    sinq_d = nc.dram_tensor("sinq", [D, SQL], F16, kind="ExternalInput")
    cosq_d = nc.dram_tensor("cosq", [D, SQL], F16, kind="ExternalInput")
    sink_d = nc.dram_tensor("sink", [D, SK], F16, kind="ExternalInput")
    cosk_d = nc.dram_tensor("cosk", [D, SK], F16, kind="ExternalInput")
    wq_d = nc.dram_tensor("wqT", [D, D], F16, kind="ExternalInput")
    wk_d = nc.dram_tensor("wkT", [D, D], F16, kind="ExternalInput")
    wv_d = nc.dram_tensor("wvT", [D, D], F16, kind="ExternalInput")
    wo_d = nc.dram_tensor("woT", [D, D], F16, kind="ExternalInput")
    bq_d = nc.dram_tensor("bqR", [128, DC], F32, kind="ExternalInput")
    bk_d = nc.dram_tensor("bkR", [128, DC], F32, kind="ExternalInput")
    bo_d = nc.dram_tensor("boR", [128, DC], F32, kind="ExternalInput")
    ones_d = nc.dram_tensor("ones128", [128, 128], F16, kind="ExternalInput")
    onesc_d = nc.dram_tensor("onescol", [128, H], F32R, kind="ExternalInput")
    out_d = nc.dram_tensor("outT", [D, SQL], F32, kind="ExternalOutput")

    rden_d = nc.dram_tensor("rden_scratch", [H, SQL], F32, kind="Internal")

    with tile.TileContext(nc) as tc:
        import contextlib
        ctx = contextlib.ExitStack()
        with ctx:
            persist = ctx.enter_context(tc.tile_pool(name="persist", bufs=1))
            ph1 = ctx.enter_context(tc.tile_pool(name="ph1", bufs=1))
            tmp = ctx.enter_context(tc.tile_pool(name="tmp", bufs=2))
            big = ctx.enter_context(tc.tile_pool(name="big", bufs=2, space="PSUM"))
            pp = ctx.enter_context(tc.tile_pool(name="pp", bufs=2, space="PSUM"))

            # ---- persistent small tensors -------------------------------
            bq_sb = persist.tile([128, DC], F32)
            bk_sb = persist.tile([128, DC], F32)
            bo_sb = persist.tile([128, DC], F32)
            ones_sb = persist.tile([128, 128], F16)
            onesc_sb = persist.tile([128, H], F32R)
            eps_t = persist.tile([128, 1], F32)
            nc.sync.dma_start(out=bq_sb, in_=bq_d[:, :])
            nc.sync.dma_start(out=bk_sb, in_=bk_d[:, :])
            nc.sync.dma_start(out=bo_sb, in_=bo_d[:, :])
            nc.sync.dma_start(out=ones_sb, in_=ones_d[:, :])
            nc.sync.dma_start(out=onesc_sb, in_=onesc_d[:, :])
            nc.vector.memset(eps_t, EPS)

            # ---- persistent activations ---------------------------------
            qrot = [persist.tile([128, SQL], F32R, name=f"qrot{c}") for c in range(DC)]
            krot = [persist.tile([128, SK], F32R, name=f"krot{c}") for c in range(DC)]
            vp = [persist.tile([128, H, HD + 1], F32R, name=f"vp{c}") for c in range(SKC)]
            oTn = [persist.tile([128, SQL], F16, name=f"oTn{c}") for c in range(DC)]
            out_sb = [persist.tile([128, SQL], F32, name=f"osb{c}") for c in range(DC)]

            # ---- phase-1 inputs -----------------------------------------
            qT = [ph1.tile([128, SQL], F16, name=f"qT{c}") for c in range(DC)]
            kvT = [ph1.tile([128, SK], F16, name=f"kvT{c}") for c in range(DC)]
            wv = [ph1.tile([128, D], F16, name=f"wv{c}") for c in range(DC)]
            for c in range(DC):
                nc.sync.dma_start(out=kvT[c], in_=kvT_d[c * 128:(c + 1) * 128, :])
                nc.sync.dma_start(out=qT[c], in_=qT_d[c * 128:(c + 1) * 128, :])
                nc.sync.dma_start(out=wv[c], in_=wv_d[c * 128:(c + 1) * 128, :])

            def wchunk(dram, c):
                t = tmp.tile([128, D], F16, tag="wchunk", bufs=2, name="wch")
                nc.sync.dma_start(out=t, in_=dram[c * 128:(c + 1) * 128, :])
                return t

            def block_swap(dst, src):
                for base in (0, 64):
                    nc.gpsimd.dma_start(out=dst[base:base + 32, :],
                                        in_=src[base + 32:base + 64, :])
                    nc.gpsimd.dma_start(out=dst[base + 32:base + 64, :],
                                        in_=src[base:base + 32, :])

            # ---- RMSNorm over features (partition dim) ------------------
            ss = pp.tile([128, SQL], F32, tag="pp")
            for c in range(DC):
                sq = tmp.tile([128, SQL], F16, tag="sq", bufs=3, name="sq")
                nc.vector.tensor_mul(out=sq, in0=qT[c], in1=qT[c])
                nc.tensor.matmul(ss, ones_sb, sq, start=(c == 0), stop=(c == DC - 1))
            lnv = tmp.tile([128, SQL], F32, tag="lnv", name="lnv")
            nc.scalar.activation(out=lnv, in_=ss, func=AF.Ln, bias=eps_t,
                                 scale=1.0 / D)
            rstd = tmp.tile([128, SQL], F32, tag="rstd", name="rstd")
            nc.scalar.activation(out=rstd, in_=lnv, func=AF.Exp, scale=-0.5)
            for c in range(DC):
                nc.vector.tensor_mul(out=qT[c], in0=qT[c], in1=rstd)

            # ---- projection + RoPE (shared for Q and K halves) ----------
            def proj_rope(w_dram, xs, xsl, bias_sb, sin_dram, cos_dram, coff,
                          dst, dsl):
                pk = [big.tile([128, 3 * SQL], F32, tag="big", name=f"pk{j}")
                      for j in range(2)]
                for c in range(DC):
                    wc = wchunk(w_dram, c)
                    for j in range(2):
                        for mm in range(3):
                            m = 3 * j + mm
                            nc.tensor.matmul(
                                pk[j][:, mm * SQL:(mm + 1) * SQL],
                                wc[:, m * 128:(m + 1) * 128], xs[c][:, xsl],
                                start=(c == 0), stop=(c == DC - 1))
                for m in range(DC):
                    j, mm = divmod(m, 3)
                    kp = tmp.tile([128, SQL], F32R, tag="kp", bufs=4, name="kp")
                    nc.scalar.activation(out=kp, in_=pk[j][:, mm * SQL:(mm + 1) * SQL],
                                         func=AF.Identity, bias=bias_sb[:, m:m + 1])
                    sin_t = tmp.tile([128, SQL], F16, tag="tbl", bufs=4, name="sin_t")
                    cos_t = tmp.tile([128, SQL], F16, tag="tbl", bufs=4, name="cos_t")
                    nc.scalar.dma_start(
                        out=sin_t, in_=sin_dram[m * 128:(m + 1) * 128,
                                                coff:coff + SQL])
                    nc.scalar.dma_start(
                        out=cos_t, in_=cos_dram[m * 128:(m + 1) * 128,
                                                coff:coff + SQL])
                    t1 = tmp.tile([128, SQL], F32R, tag="sw", bufs=4, name="t1")
                    block_swap(t1, kp)
                    nc.vector.tensor_mul(out=t1, in0=t1, in1=sin_t)
                    nc.vector.tensor_mul(out=dst[m][:, dsl], in0=kp, in1=cos_t)
                    nc.vector.tensor_add(out=dst[m][:, dsl], in0=dst[m][:, dsl],
                                         in1=t1)

            for half in range(2):
                hs = slice(half * SQL, (half + 1) * SQL)
                proj_rope(wk_d, kvT, hs, bk_sb, sink_d, cosk_d, half * SQL,
                          krot, hs)
            proj_rope(wq_d, qT, slice(0, SQL), bq_sb, sinq_d, cosq_d, 0,
                      qrot, slice(0, SQL))

            # ---- V projection (row-major, ones column appended) ---------
            for kc in range(SKC):
                ksl = slice(kc * 128, (kc + 1) * 128)
                pv = big.tile([128, 3 * SQL], F32, tag="big", name="pv")
                for c in range(DC):
                    nc.tensor.matmul(pv[:, 0:512], kvT[c][:, ksl],
                                     wv[c][:, 0:512],
                                     start=(c == 0), stop=(c == DC - 1))
                    nc.tensor.matmul(pv[:, 512:768], kvT[c][:, ksl],
                                     wv[c][:, 512:768],
                                     start=(c == 0), stop=(c == DC - 1))
                nc.vector.tensor_copy(
                    out=vp[kc][:, 0:8, 0:HD],
                    in_=pv[:, 0:512].rearrange("p (h d) -> p h d", h=8))
                nc.vector.tensor_copy(
                    out=vp[kc][:, 8:12, 0:HD],
                    in_=pv[:, 512:768].rearrange("p (h d) -> p h d", h=4))
                nc.gpsimd.dma_start(out=vp[kc][:, :, HD], in_=onesc_sb)

            # ---- attention + pipelined normalize/out-proj ---------------
            import concourse.bass as bass_mod
            GROUPS = [(0, 3), (3, 6), (6, 8)]
            for h in range(H):
                mh, off = h // 2, 64 * (h % 2)
                po = pp.tile([65, SQL], F32, tag="pp", name="po")
                for k0, k1 in GROUPS:
                    w = (k1 - k0) * SQL
                    sc = big.tile([128, 3 * SQL], F32, tag="big", name="sc")
                    for i, kc in enumerate(range(k0, k1)):
                        nc.tensor.matmul(
                            sc[:, i * SQL:(i + 1) * SQL],
                            krot[mh][off:off + 64, kc * 128:(kc + 1) * 128],
                            qrot[mh][off:off + 64, :],
                            start=True, stop=True)
                    ex = tmp.tile([128, 3 * SQL], F32R, tag="ex", bufs=3,
                                  name="ex")
                    nc.scalar.activation(out=ex[:, :w], in_=sc[:, :w],
                                         func=AF.Exp, scale=1.0 / 8.0)
                    for i, kc in enumerate(range(k0, k1)):
                        nc.tensor.matmul(po, vp[kc][:, h, :],
                                         ex[:, i * SQL:(i + 1) * SQL],
                                         start=(kc == 0), stop=(kc == SKC - 1))
                # denominator: reciprocal straight off PSUM, DRAM broadcast
                rden = tmp.tile([1, SQL], F32, tag="rden", bufs=3, name="rden")
                nc.vector.reciprocal(out=rden, in_=po[64:65, :])
                nc.sync.dma_start(out=rden_d[h:h + 1, :], in_=rden)
                rb = tmp.tile([64, SQL], F32, tag="rb", bufs=3, name="rb")
                row = rden_d[h, :]
                bsrc = bass_mod.AP(tensor=row.tensor, offset=row.offset,
                                   ap=[[0, 64], *row.ap])
                nc.gpsimd.dma_start(out=rb, in_=bsrc)
                nc.vector.tensor_mul(out=oTn[mh][off:off + 64, :],
                                     in0=po[0:64, :], in1=rb)

                if h % 2 == 1:
                    # head pair (2*mh, 2*mh+1) done: fold contraction chunk
                    # mh of the output projection into SBUF accumulators
                    woc = wchunk(wo_d, mh)
                    for j in range(2):
                        pob = big.tile([128, 3 * SQL], F32, tag="big",
                                       name="pob")
                        for mm in range(3):
                            m = 3 * j + mm
                            nc.tensor.matmul(pob[:, mm * SQL:(mm + 1) * SQL],
                                             woc[:, m * 128:(m + 1) * 128],
                                             oTn[mh], start=True, stop=True)
                        for mm in range(3):
                            m = 3 * j + mm
                            psl = pob[:, mm * SQL:(mm + 1) * SQL]
                            if mh == 0:
                                nc.vector.tensor_scalar_add(
                                    out=out_sb[m], in0=psl,
                                    scalar1=bo_sb[:, m:m + 1])
                            else:
                                nc.vector.tensor_add(out=out_sb[m],
                                                     in0=out_sb[m], in1=psl)

            for m in range(DC):
                nc.sync.dma_start(out=out_d[m * 128:(m + 1) * 128, :],
                                  in_=out_sb[m])

    return nc


# ---------------------------------------------------------------------------
# host wrapper
# ---------------------------------------------------------------------------

def _rope_tables(pos, freqs):
    """pos [S,2] int, freqs [2,H,32] -> signed-sin and cos tables [D,S] fp16,
    rows in de-interleaved head-dim order (pair j at rows h*64+j / h*64+32+j,
    first-half sin rows negated so qrot = x*cos + swap(x)*sin)."""
    ang = np.einsum('sd,dhj->hjs', pos.astype(np.float64),
                    freqs.astype(np.float64))          # [H,32,S]
    s, c = np.sin(ang), np.cos(ang)
    sin_full = np.concatenate([-s, s], axis=1).reshape(D, -1)
    cos_full = np.concatenate([c, c], axis=1).reshape(D, -1)
    return sin_full.astype(np.float16), cos_full.astype(np.float16)


def kernel(q, kv, posq, posk, w_norm, w_q, b_q, w_kv, b_kv, w_out, b_out, freqs):
    _apply_patches()
    from concourse.bass_utils import run_bass_kernel_spmd

    q = np.asarray(q, np.float32)
    kv = np.asarray(kv, np.float32)
    posq_np = np.asarray(posq)
    posk_np = np.asarray(posk)
    w_norm = np.asarray(w_norm, np.float32)
    w_q = np.asarray(w_q, np.float32)
    b_q = np.asarray(b_q, np.float32)
    w_kv = np.asarray(w_kv, np.float32)
    b_kv = np.asarray(b_kv, np.float32)
    w_out = np.asarray(w_out, np.float32)
    b_out = np.asarray(b_out, np.float32)
    freqs = np.asarray(freqs, np.float32)

    # de-interleave head dims: new j<32 -> old 2j (even), j>=32 -> old 2(j-32)+1
    perm = np.empty(D, np.int64)
    for h in range(H):
        for j in range(HD):
            perm[h * HD + j] = h * HD + (2 * j if j < 32 else 2 * (j - 32) + 1)

    wqT = np.ascontiguousarray((w_q[perm, :] * w_norm[None, :]).T).astype(np.float16)
    wkT = np.ascontiguousarray(w_kv[:D][perm, :].T).astype(np.float16)
    wvT = np.ascontiguousarray(w_kv[D:].T).astype(np.float16)
    woT = np.ascontiguousarray(w_out.T).astype(np.float16)
    bqR = np.ascontiguousarray(b_q[perm].reshape(DC, 128).T)
    bkR = np.ascontiguousarray(b_kv[:D][perm].reshape(DC, 128).T)
    bo_eff = b_out + w_out @ b_kv[D:]          # fold V bias (softmax sums to 1)
    boR = np.ascontiguousarray(bo_eff.reshape(DC, 128).T)

    ones128 = np.ones((128, 128), np.float16)
    onescol = np.ones((128, H), np.float32)

    if "nc" not in _cache:
        _cache["nc"] = _build_nc()
    nc = _cache["nc"]

    in_maps = []
    for core in range(NCORES):
        b, half = core // 2, core % 2
        qs = slice(half * SQL, (half + 1) * SQL)
        sinq, cosq = _rope_tables(posq_np[b], freqs)
        sink, cosk = _rope_tables(posk_np[b], freqs)
        in_maps.append({
            "qT": np.ascontiguousarray(q[b, qs, :].T).astype(np.float16),
            "kvT": np.ascontiguousarray(kv[b].T).astype(np.float16),
            "sinq": np.ascontiguousarray(sinq[:, qs]),
            "cosq": np.ascontiguousarray(cosq[:, qs]),
            "sink": np.ascontiguousarray(sink),
            "cosk": np.ascontiguousarray(cosk),
            "wqT": wqT, "wkT": wkT, "wvT": wvT, "woT": woT,
            "bqR": bqR, "bkR": bkR, "boR": boR,
            "ones128": ones128, "onescol": onescol,
        })

    res = run_bass_kernel_spmd(nc, in_maps, core_ids=list(range(NCORES)))
    kernel._last_result = res

    out = np.empty((B, SQ, D), np.float32)
    for core in range(NCORES):
        b, half = core // 2, core % 2
        out[b, half * SQL:(half + 1) * SQL, :] = res.results[core]["outT"].T
    return out


# revision 9
# speedup vs baseline: 1.4213x; 1.2097x over previous
"""Cross-attention (RMSNorm + QKV proj + 2D RoPE + SDPA + out-proj) on 8
Trainium2 NeuronCores.

Sharding: 8 cores = 4 batches x 2 query-halves. Each core computes the full
KV projection for its batch (duplicated across the 2 cores sharing a batch)
and attention + output projection for its 512 query rows. No collectives.

On-device layout is feature-major: activations live as [feature, seq] with
features on SBUF partitions. Host pre-transposes inputs and weights (fp16)
so every linear layer is a plain lhsT.T @ rhs PE matmul at full rate. Head
dims are de-interleaved (even rot dims then odd rot dims per head) so RoPE's
pair rotation becomes a 32-partition block swap plus two multiplies against
host-precomputed sin/cos tables (fp16, sign folded into the sin rows).

RMSNorm's rsqrt is computed as exp(-0.5*ln(x)) so the whole kernel needs a
single ACT table set (natural_log_exp). Softmax skips max-subtraction
(logits are O(1) by construction); the denominator comes free as a 65th
ones-column in the AV matmul. Scores for three key chunks accumulate into a
3-bank PSUM tile so each softmax exp covers N=1536 at once. The per-head
denominator reciprocal + broadcast and the per-head-pair output projection
(accumulated in SBUF) pipeline behind later heads' attention.
"""

import numpy as np

B, SQ, SK, D = 4, 1024, 1024, 768
H, HD = 12, 64
DC = D // 128          # 6 feature chunks
SQL = SQ // 2          # 512 query rows per core
SKC = SK // 128        # 8 key chunks
EPS = 1e-5
NCORES = 8

_cache = {}


# ---------------------------------------------------------------------------
# compiler workarounds
# ---------------------------------------------------------------------------

def _apply_patches():
    """This walrus build allows only ONE sync-wait command per instruction.
    (a) split the Tile kernel-tail drain into one drain per waited proc;
    (b) post-process the BIR JSON, moving excess waits onto same-engine NoOps
    inserted immediately before the over-subscribed instruction."""
    import json
    import concourse.tile as tile
    import concourse.bass as cbass
    from concourse.vector_clock import ScopedClock, VectorClock

    if getattr(cbass.Bass, "_wait_split_patched", False):
        return

    def _drain_and_barrier(self, tick_clock, wait_clock):
        gc = tick_clock.global_clock
        try:
            vec = gc[None]
        except Exception:
            vec = gc
        n = len(vec)
        for p in [i for i in range(n) if vec[i] > 0]:
            sub = [0] * n
            sub[p] = vec[p]
            inst = self.nc.sync.drain()
            wait_clock.add_sem_waits(inst.ins, ScopedClock({None: VectorClock(sub)}))
        self.nc.all_engine_barrier()
        assert self.sems is not None
        popped = self.nc._tile_sem_poison_stack.pop()
        assert popped is self._sem_poison
        self.nc.clear_and_free_semaphores(list(self.sems.allocated().values()))
        self.nc.all_engine_barrier()

    tile.TileContext._drain_and_barrier = _drain_and_barrier

    def _split_waits(bir):
        for f in bir.get("functions", []):
            for blk in f.get("blocks", []):
                insts = blk.get("instructions")
                if not insts:
                    continue
                out = []
                ctr = 0
                for inst in insts:
                    si = inst.get("sync_info")
                    ow = (si or {}).get("on_wait") or []
                    if len(ow) > 1:
                        for w in ow[:-1]:
                            nop = {
                                "name": f"{inst['name']}-ws{ctr}",
                                "opcode": "NoOp",
                                "engine": inst.get("engine"),
                                "ins": [],
                                "outs": [],
                                "sync_info": {"on_wait": [w], "on_update": []},
                            }
                            if "debug" in inst:
                                nop["debug"] = inst["debug"]
                            ctr += 1
                            out.append(nop)
                        si["on_wait"] = [ow[-1]]
                    out.append(inst)
                blk["instructions"] = out
        return bir

    orig = cbass.Bass.to_json_bytes

    def to_json_bytes(self, *a, **kw):
        return json.dumps(_split_waits(json.loads(orig(self, *a, **kw)))).encode()

    cbass.Bass.to_json_bytes = to_json_bytes
    cbass.Bass._wait_split_patched = True


# ---------------------------------------------------------------------------
# device program
# ---------------------------------------------------------------------------

def _build_nc():
    import concourse.bass as bass
    import concourse.tile as tile
    import concourse.mybir as mybir

    F32 = mybir.dt.float32
    F32R = mybir.dt.float32r
    F16 = mybir.dt.float16
    AF = mybir.ActivationFunctionType
    ALU = mybir.AluOpType

    nc = bass.Bass()

    qT_d = nc.dram_tensor("qT", [D, SQL], F16, kind="ExternalInput")
    kvT_d = nc.dram_tensor("kvT", [D, SK], F16, kind="ExternalInput")
    sinq_d = nc.dram_tensor("sinq", [D, SQL], F16, kind="ExternalInput")
    cosq_d = nc.dram_tensor("cosq", [D, SQL], F16, kind="ExternalInput")
    sink_d = nc.dram_tensor("sink", [D, SK], F16, kind="ExternalInput")
    cosk_d = nc.dram_tensor("cosk", [D, SK], F16, kind="ExternalInput")
    wq_d = nc.dram_tensor("wqT", [D, D], F16, kind="ExternalInput")
    wk_d = nc.dram_tensor("wkT", [D, D], F16, kind="ExternalInput")
    wv_d = nc.dram_tensor("wvT", [D, D], F16, kind="ExternalInput")
    wo_d = nc.dram_tensor("woT", [D, D], F16, kind="ExternalInput")
    bq_d = nc.dram_tensor("bqR", [128, DC], F32, kind="ExternalInput")
    bk_d = nc.dram_tensor("bkR", [128, DC], F32, kind="ExternalInput")
    bo_d = nc.dram_tensor("boR", [128, DC], F32, kind="ExternalInput")
    ones_d = nc.dram_tensor("ones128", [128, 128], F16, kind="ExternalInput")
    onesc_d = nc.dram_tensor("onescol", [128, H], F32R, kind="ExternalInput")
    out_d = nc.dram_tensor("outT", [D, SQL], F32, kind="ExternalOutput")

    rden_d = nc.dram_tensor("rden_scratch", [H, SQL], F32, kind="Internal")

    with tile.TileContext(nc) as tc:
        import contextlib
        ctx = contextlib.ExitStack()
        with ctx:
            persist = ctx.enter_context(tc.tile_pool(name="persist", bufs=1))
            ph1 = ctx.enter_context(tc.tile_pool(name="ph1", bufs=1))
            tmp = ctx.enter_context(tc.tile_pool(name="tmp", bufs=2))
            big = ctx.enter_context(tc.tile_pool(name="big", bufs=2, space="PSUM"))
            pp = ctx.enter_context(tc.tile_pool(name="pp", bufs=2, space="PSUM"))

            # ---- persistent small tensors -------------------------------
            bq_sb = persist.tile([128, DC], F32)
            bk_sb = persist.tile([128, DC], F32)
            bo_sb = persist.tile([128, DC], F32)
            ones_sb = persist.tile([128, 128], F16)
            onesc_sb = persist.tile([128, H], F32R)
            eps_t = persist.tile([128, 1], F32)
            nc.gpsimd.dma_start(out=ones_sb, in_=ones_d[:, :])
            nc.gpsimd.dma_start(out=bq_sb, in_=bq_d[:, :])
            nc.gpsimd.dma_start(out=bk_sb, in_=bk_d[:, :])
            nc.gpsimd.dma_start(out=bo_sb, in_=bo_d[:, :])
            nc.gpsimd.dma_start(out=onesc_sb, in_=onesc_d[:, :])
            nc.vector.memset(eps_t, EPS)

            # ---- persistent activations ---------------------------------
            qrot = [persist.tile([128, SQL], F32R, name=f"qrot{c}") for c in range(DC)]
            krot = [persist.tile([128, SK], F32R, name=f"krot{c}") for c in range(DC)]
            vp = [persist.tile([128, H, HD + 1], F32R, name=f"vp{c}") for c in range(SKC)]
            oTn = [persist.tile([128, SQL], F16, name=f"oTn{c}") for c in range(DC)]
            out_sb = [persist.tile([128, SQL], F32, name=f"osb{c}") for c in range(DC)]

            # ---- phase-1 inputs -----------------------------------------
            qT = [ph1.tile([128, SQL], F16, name=f"qT{c}") for c in range(DC)]
            kvT = [ph1.tile([128, SK], F16, name=f"kvT{c}") for c in range(DC)]
            wv = [ph1.tile([128, D], F16, name=f"wv{c}") for c in range(DC)]
            # spread input streams over independent DMA queues so they load
            # in parallel: kvT on sync, qT + weight chunks on scalar,
            # wv on gpsimd
            for c in range(DC):
                nc.sync.dma_start(out=kvT[c], in_=kvT_d[c * 128:(c + 1) * 128, :])
                nc.scalar.dma_start(out=qT[c], in_=qT_d[c * 128:(c + 1) * 128, :])
                nc.gpsimd.dma_start(out=wv[c], in_=wv_d[c * 128:(c + 1) * 128, :])

            def wchunk(dram, c):
                t = tmp.tile([128, D], F16, tag="wchunk", bufs=3, name="wch")
                nc.scalar.dma_start(out=t, in_=dram[c * 128:(c + 1) * 128, :])
                return t

            def block_swap(dst, src):
                for base in (0, 64):
                    nc.gpsimd.dma_start(out=dst[base:base + 32, :],
                                        in_=src[base + 32:base + 64, :])
                    nc.gpsimd.dma_start(out=dst[base + 32:base + 64, :],
                                        in_=src[base:base + 32, :])

            # ---- RMSNorm over features (partition dim) ------------------
            ss = pp.tile([128, SQL], F32, tag="pp")
            for c in range(DC):
                sq = tmp.tile([128, SQL], F16, tag="sq", bufs=3, name="sq")
                nc.vector.tensor_mul(out=sq, in0=qT[c], in1=qT[c])
                nc.tensor.matmul(ss, ones_sb, sq, start=(c == 0), stop=(c == DC - 1))
            lnv = tmp.tile([128, SQL], F32, tag="lnv", name="lnv")
            nc.scalar.activation(out=lnv, in_=ss, func=AF.Ln, bias=eps_t,
                                 scale=1.0 / D)
            rstd = tmp.tile([128, SQL], F32, tag="rstd", name="rstd")
            nc.scalar.activation(out=rstd, in_=lnv, func=AF.Exp, scale=-0.5)
            for c in range(DC):
                nc.vector.tensor_mul(out=qT[c], in0=qT[c], in1=rstd)

            # ---- projection + RoPE (shared for Q and K halves) ----------
            def proj_rope(w_dram, xs, xsl, bias_sb, sin_dram, cos_dram, coff,
                          dst, dsl):
                pk = [big.tile([128, 3 * SQL], F32, tag="big", name=f"pk{j}")
                      for j in range(2)]
                for c in range(DC):
                    wc = wchunk(w_dram, c)
                    for j in range(2):
                        for mm in range(3):
                            m = 3 * j + mm
                            nc.tensor.matmul(
                                pk[j][:, mm * SQL:(mm + 1) * SQL],
                                wc[:, m * 128:(m + 1) * 128], xs[c][:, xsl],
                                start=(c == 0), stop=(c == DC - 1))
                for m in range(DC):
                    j, mm = divmod(m, 3)
                    kp = tmp.tile([128, SQL], F32R, tag="kp", bufs=4, name="kp")
                    nc.vector.tensor_scalar_add(
                        out=kp, in0=pk[j][:, mm * SQL:(mm + 1) * SQL],
                        scalar1=bias_sb[:, m:m + 1])
                    sin_t = tmp.tile([128, SQL], F16, tag="tbl", bufs=4, name="sin_t")
                    cos_t = tmp.tile([128, SQL], F16, tag="tbl", bufs=4, name="cos_t")
                    nc.sync.dma_start(
                        out=sin_t, in_=sin_dram[m * 128:(m + 1) * 128,
                                                coff:coff + SQL])
                    nc.sync.dma_start(
                        out=cos_t, in_=cos_dram[m * 128:(m + 1) * 128,
                                                coff:coff + SQL])
                    t1 = tmp.tile([128, SQL], F32R, tag="sw", bufs=4, name="t1")
                    block_swap(t1, kp)
                    nc.vector.tensor_mul(out=t1, in0=t1, in1=sin_t)
                    nc.vector.tensor_mul(out=dst[m][:, dsl], in0=kp, in1=cos_t)
                    nc.vector.tensor_add(out=dst[m][:, dsl], in0=dst[m][:, dsl],
                                         in1=t1)

            for half in range(2):
                hs = slice(half * SQL, (half + 1) * SQL)
                proj_rope(wk_d, kvT, hs, bk_sb, sink_d, cosk_d, half * SQL,
                          krot, hs)
            proj_rope(wq_d, qT, slice(0, SQL), bq_sb, sinq_d, cosq_d, 0,
                      qrot, slice(0, SQL))

            # ---- V projection (row-major, ones column appended) ---------
            for kc in range(SKC):
                ksl = slice(kc * 128, (kc + 1) * 128)
                pv = big.tile([128, 3 * SQL], F32, tag="big", name="pv")
                for c in range(DC):
                    nc.tensor.matmul(pv[:, 0:512], kvT[c][:, ksl],
                                     wv[c][:, 0:512],
                                     start=(c == 0), stop=(c == DC - 1))
                    nc.tensor.matmul(pv[:, 512:768], kvT[c][:, ksl],
                                     wv[c][:, 512:768],
                                     start=(c == 0), stop=(c == DC - 1))
                nc.vector.tensor_copy(
                    out=vp[kc][:, 0:8, 0:HD],
                    in_=pv[:, 0:512].rearrange("p (h d) -> p h d", h=8))
                nc.vector.tensor_copy(
                    out=vp[kc][:, 8:12, 0:HD],
                    in_=pv[:, 512:768].rearrange("p (h d) -> p h d", h=4))
                nc.gpsimd.dma_start(out=vp[kc][:, :, HD], in_=onesc_sb)

            # ---- attention + pipelined normalize/out-proj ---------------
            import concourse.bass as bass_mod
            GROUPS = [(0, 3), (3, 6), (6, 8)]
            for h in range(H):
                mh, off = h // 2, 64 * (h % 2)
                po = pp.tile([65, SQL], F32, tag="pp", name="po")
                for k0, k1 in GROUPS:
                    w = (k1 - k0) * SQL
                    sc = big.tile([128, 3 * SQL], F32, tag="big", name="sc")
                    for i, kc in enumerate(range(k0, k1)):
                        nc.tensor.matmul(
                            sc[:, i * SQL:(i + 1) * SQL],
                            krot[mh][off:off + 64, kc * 128:(kc + 1) * 128],
                            qrot[mh][off:off + 64, :],
                            start=True, stop=True)
                    ex = tmp.tile([128, 3 * SQL], F32R, tag="ex", bufs=3,
                                  name="ex")
                    nc.scalar.activation(out=ex[:, :w], in_=sc[:, :w],
                                         func=AF.Exp, scale=1.0 / 8.0)
                    for i, kc in enumerate(range(k0, k1)):
                        nc.tensor.matmul(po, vp[kc][:, h, :],
                                         ex[:, i * SQL:(i + 1) * SQL],
                                         start=(kc == 0), stop=(kc == SKC - 1))
                # denominator: reciprocal straight off PSUM, DRAM broadcast
                rden = tmp.tile([1, SQL], F32, tag="rden", bufs=3, name="rden")
                nc.vector.reciprocal(out=rden, in_=po[64:65, :])
                nc.sync.dma_start(out=rden_d[h:h + 1, :], in_=rden)
                rb = tmp.tile([64, SQL], F32, tag="rb", bufs=3, name="rb")
                row = rden_d[h, :]
                bsrc = bass_mod.AP(tensor=row.tensor, offset=row.offset,
                                   ap=[[0, 64], *row.ap])
                nc.gpsimd.dma_start(out=rb, in_=bsrc)
                nc.vector.tensor_mul(out=oTn[mh][off:off + 64, :],
                                     in0=po[0:64, :], in1=rb)

            # ---- output projection tail (c-outer, 2 psum accumulators) --
            po6 = [big.tile([128, 3 * SQL], F32, tag="big", name=f"po6_{j}")
                   for j in range(2)]
            for c in range(DC):
                woc = wchunk(wo_d, c)
                for j in range(2):
                    for mm in range(3):
                        m = 3 * j + mm
                        nc.tensor.matmul(po6[j][:, mm * SQL:(mm + 1) * SQL],
                                         woc[:, m * 128:(m + 1) * 128],
                                         oTn[c], start=(c == 0),
                                         stop=(c == DC - 1))
            for m in range(DC):
                j, mm = divmod(m, 3)
                nc.vector.tensor_scalar_add(
                    out=out_sb[m], in0=po6[j][:, mm * SQL:(mm + 1) * SQL],
                    scalar1=bo_sb[:, m:m + 1])
                nc.sync.dma_start(out=out_d[m * 128:(m + 1) * 128, :],
                                  in_=out_sb[m])

    return nc


# ---------------------------------------------------------------------------
# host wrapper
# ---------------------------------------------------------------------------

def _rope_tables(pos, freqs):
    """pos [S,2] int, freqs [2,H,32] -> signed-sin and cos tables [D,S] fp16,
    rows in de-interleaved head-dim order (pair j at rows h*64+j / h*64+32+j,
    first-half sin rows negated so qrot = x*cos + swap(x)*sin)."""
    ang = np.einsum('sd,dhj->hjs', pos.astype(np.float64),
                    freqs.astype(np.float64))          # [H,32,S]
    s, c = np.sin(ang), np.cos(ang)
    sin_full = np.concatenate([-s, s], axis=1).reshape(D, -1)
    cos_full = np.concatenate([c, c], axis=1).reshape(D, -1)
    return sin_full.astype(np.float16), cos_full.astype(np.float16)


def kernel(q, kv, posq, posk, w_norm, w_q, b_q, w_kv, b_kv, w_out, b_out, freqs):
    _apply_patches()
    from concourse.bass_utils import run_bass_kernel_spmd

    q = np.asarray(q, np.float32)
    kv = np.asarray(kv, np.float32)
    posq_np = np.asarray(posq)
    posk_np = np.asarray(posk)
    w_norm = np.asarray(w_norm, np.float32)
    w_q = np.asarray(w_q, np.float32)
    b_q = np.asarray(b_q, np.float32)
    w_kv = np.asarray(w_kv, np.float32)
    b_kv = np.asarray(b_kv, np.float32)
    w_out = np.asarray(w_out, np.float32)
    b_out = np.asarray(b_out, np.float32)
    freqs = np.asarray(freqs, np.float32)

    # de-interleave head dims: new j<32 -> old 2j (even), j>=32 -> old 2(j-32)+1
    perm = np.empty(D, np.int64)
    for h in range(H):
        for j in range(HD):
            perm[h * HD + j] = h * HD + (2 * j if j < 32 else 2 * (j - 32) + 1)

    wqT = np.ascontiguousarray((w_q[perm, :] * w_norm[None, :]).T).astype(np.float16)
    wkT = np.ascontiguousarray(w_kv[:D][perm, :].T).astype(np.float16)
    wvT = np.ascontiguousarray(w_kv[D:].T).astype(np.float16)
    woT = np.ascontiguousarray(w_out.T).astype(np.float16)
    bqR = np.ascontiguousarray(b_q[perm].reshape(DC, 128).T)
    bkR = np.ascontiguousarray(b_kv[:D][perm].reshape(DC, 128).T)
    bo_eff = b_out + w_out @ b_kv[D:]          # fold V bias (softmax sums to 1)
    boR = np.ascontiguousarray(bo_eff.reshape(DC, 128).T)

    ones128 = np.ones((128, 128), np.float16)
    onescol = np.ones((128, H), np.float32)

    if "nc" not in _cache:
        _cache["nc"] = _build_nc()
    nc = _cache["nc"]

    in_maps = []
    for core in range(NCORES):
        b, half = core // 2, core % 2
        qs = slice(half * SQL, (half + 1) * SQL)
        sinq, cosq = _rope_tables(posq_np[b], freqs)
        sink, cosk = _rope_tables(posk_np[b], freqs)
        in_maps.append({
            "qT": np.ascontiguousarray(q[b, qs, :].T).astype(np.float16),
            "kvT": np.ascontiguousarray(kv[b].T).astype(np.float16),
            "sinq": np.ascontiguousarray(sinq[:, qs]),
            "cosq": np.ascontiguousarray(cosq[:, qs]),
            "sink": np.ascontiguousarray(sink),
            "cosk": np.ascontiguousarray(cosk),
            "wqT": wqT, "wkT": wkT, "wvT": wvT, "woT": woT,
            "bqR": bqR, "bkR": bkR, "boR": boR,
            "ones128": ones128, "onescol": onescol,
        })

    res = run_bass_kernel_spmd(nc, in_maps, core_ids=list(range(NCORES)))
    kernel._last_result = res

    out = np.empty((B, SQ, D), np.float32)
    for core in range(NCORES):
        b, half = core // 2, core % 2
        out[b, half * SQL:(half + 1) * SQL, :] = res.results[core]["outT"].T
    return out


# revision 13
# speedup vs baseline: 1.5852x; 1.1154x over previous
"""Cross-attention (RMSNorm + QKV proj + 2D RoPE + SDPA + out-proj) on 8
Trainium2 NeuronCores.

Sharding: 8 cores = 4 batches x 2 query-halves. Each core computes the full
KV projection for its batch (duplicated across the 2 cores sharing a batch)
and attention + output projection for its 512 query rows. No collectives.

On-device layout is feature-major: activations live as [feature, seq] with
features on SBUF partitions. Host pre-transposes inputs and weights (fp16)
so every linear layer is a plain lhsT.T @ rhs PE matmul at full rate. Head
dims are de-interleaved (even rot dims then odd rot dims per head) so RoPE's
pair rotation becomes a 32-partition block swap plus two fused
(bias-add)*table multiplies against host-precomputed sin/cos tables (fp16,
sign folded into the sin rows).

Every matmul keeps the full 128x128 array busy so the PE clock-gate (HAM)
stays at full rate: attention operands are bf16 with K for both heads of a
chunk packed on the contraction dim and the per-head Q zero-padded on its
unused 64 partitions; V tiles are padded to 128 columns (ones column at 64
for the free softmax denominator, zeros above). Projections run m-outer
with all six weight chunks resident so each PSUM accumulator retires after
six back-to-back matmuls. RMSNorm's rsqrt is exp(-0.5*ln(x)) so one ACT
table set serves the whole kernel; softmax skips max-subtraction and each
exp covers a 3-bank PSUM group (N=1536). The per-head denominator
reciprocal runs straight off PSUM with a DRAM-broadcast roundtrip, and the
output projection is a dense c-outer tail.
"""

import numpy as np

B, SQ, SK, D = 4, 1024, 1024, 768
H, HD = 12, 64
DC = D // 128          # 6 feature chunks
SQL = SQ // 2          # 512 query rows per core
SKC = SK // 128        # 8 key chunks
EPS = 1e-5
NCORES = 8

_cache = {}


# ---------------------------------------------------------------------------
# compiler workarounds
# ---------------------------------------------------------------------------

def _apply_patches():
    """This walrus build allows only ONE sync-wait command per instruction.
    (a) split the Tile kernel-tail drain into one drain per waited proc;
    (b) post-process the BIR JSON, moving excess waits onto same-engine NoOps
    inserted immediately before the over-subscribed instruction."""
    import json
    import concourse.tile as tile
    import concourse.bass as cbass
    from concourse.vector_clock import ScopedClock, VectorClock

    if getattr(cbass.Bass, "_wait_split_patched", False):
        return

    def _drain_and_barrier(self, tick_clock, wait_clock):
        gc = tick_clock.global_clock
        try:
            vec = gc[None]
        except Exception:
            vec = gc
        n = len(vec)
        for p in [i for i in range(n) if vec[i] > 0]:
            sub = [0] * n
            sub[p] = vec[p]
            inst = self.nc.sync.drain()
            wait_clock.add_sem_waits(inst.ins, ScopedClock({None: VectorClock(sub)}))
        self.nc.all_engine_barrier()
        assert self.sems is not None
        popped = self.nc._tile_sem_poison_stack.pop()
        assert popped is self._sem_poison
        self.nc.clear_and_free_semaphores(list(self.sems.allocated().values()))
        self.nc.all_engine_barrier()

    tile.TileContext._drain_and_barrier = _drain_and_barrier

    def _split_waits(bir):
        for f in bir.get("functions", []):
            for blk in f.get("blocks", []):
                insts = blk.get("instructions")
                if not insts:
                    continue
                out = []
                ctr = 0
                for inst in insts:
                    si = inst.get("sync_info")
                    ow = (si or {}).get("on_wait") or []
                    if len(ow) > 1:
                        for w in ow[:-1]:
                            nop = {
                                "name": f"{inst['name']}-ws{ctr}",
                                "opcode": "NoOp",
                                "engine": inst.get("engine"),
                                "ins": [],
                                "outs": [],
                                "sync_info": {"on_wait": [w], "on_update": []},
                            }
                            if "debug" in inst:
                                nop["debug"] = inst["debug"]
                            ctr += 1
                            out.append(nop)
                        si["on_wait"] = [ow[-1]]
                    out.append(inst)
                blk["instructions"] = out
        return bir

    orig = cbass.Bass.to_json_bytes

    def to_json_bytes(self, *a, **kw):
        return json.dumps(_split_waits(json.loads(orig(self, *a, **kw)))).encode()

    cbass.Bass.to_json_bytes = to_json_bytes
    cbass.Bass._wait_split_patched = True


# ---------------------------------------------------------------------------
# device program
# ---------------------------------------------------------------------------

def _build_nc():
    import concourse.bass as bass
    import concourse.tile as tile
    import concourse.mybir as mybir

    F32 = mybir.dt.float32
    F32R = mybir.dt.float32r
    F16 = mybir.dt.float16
    BF16 = mybir.dt.bfloat16
    AF = mybir.ActivationFunctionType
    ALU = mybir.AluOpType

    nc = bass.Bass()

    qT_d = nc.dram_tensor("qT", [D, SQL], F16, kind="ExternalInput")
    kvT_d = nc.dram_tensor("kvT", [D, SK], F16, kind="ExternalInput")
    sinq_d = nc.dram_tensor("sinq", [D, SQL], F16, kind="ExternalInput")
    cosq_d = nc.dram_tensor("cosq", [D, SQL], F16, kind="ExternalInput")
    sink_d = nc.dram_tensor("sink", [D, SK], F16, kind="ExternalInput")
    cosk_d = nc.dram_tensor("cosk", [D, SK], F16, kind="ExternalInput")
    wq_d = nc.dram_tensor("wqT", [D, D], F16, kind="ExternalInput")
    wk_d = nc.dram_tensor("wkT", [D, D], F16, kind="ExternalInput")
    wv_d = nc.dram_tensor("wvT", [D, D], F16, kind="ExternalInput")
    wo_d = nc.dram_tensor("woT", [D, D], BF16, kind="ExternalInput")
    bq_d = nc.dram_tensor("bqR", [128, DC], F32, kind="ExternalInput")
    bk_d = nc.dram_tensor("bkR", [128, DC], F32, kind="ExternalInput")
    bo_d = nc.dram_tensor("boR", [128, DC], F32, kind="ExternalInput")
    ones_d = nc.dram_tensor("ones128", [128, 128], F16, kind="ExternalInput")
    onesc_d = nc.dram_tensor("onescol", [128, H], BF16, kind="ExternalInput")
    out_d = nc.dram_tensor("outT", [D, SQL], F32, kind="ExternalOutput")

    rden_d = nc.dram_tensor("rden_scratch", [H, SQL], F32, kind="Internal")

    with tile.TileContext(nc) as tc:
        import contextlib
        ctx = contextlib.ExitStack()
        with ctx:
            persist = ctx.enter_context(tc.tile_pool(name="persist", bufs=1))
            ph1 = ctx.enter_context(tc.tile_pool(name="ph1", bufs=1))
            tmp = ctx.enter_context(tc.tile_pool(name="tmp", bufs=2))
            big = ctx.enter_context(tc.tile_pool(name="big", bufs=2, space="PSUM"))
            pp = ctx.enter_context(tc.tile_pool(name="pp", bufs=2, space="PSUM"))

            # ---- persistent small tensors (gpsimd SWDGE queue) ----------
            bq_sb = persist.tile([128, DC], F32)
            bk_sb = persist.tile([128, DC], F32)
            bo_sb = persist.tile([128, DC], F32)
            ones_sb = persist.tile([128, 128], F16)
            onesc_sb = persist.tile([128, H], BF16)
            eps_t = persist.tile([128, 1], F32)
            nc.gpsimd.dma_start(out=ones_sb, in_=ones_d[:, :])
            nc.gpsimd.dma_start(out=bq_sb, in_=bq_d[:, :])
            nc.gpsimd.dma_start(out=bk_sb, in_=bk_d[:, :])
            nc.gpsimd.dma_start(out=bo_sb, in_=bo_d[:, :])
            nc.gpsimd.dma_start(out=onesc_sb, in_=onesc_d[:, :])
            nc.vector.memset(eps_t, EPS)

            # ---- persistent activations ---------------------------------
            qrz = [persist.tile([128, SQL], BF16, name=f"qrz{h}") for h in range(H)]
            krot = [persist.tile([128, SK], BF16, name=f"krot{c}") for c in range(DC)]
            vp = [persist.tile([128, H, 128], BF16, name=f"vp{c}") for c in range(SKC)]
            oTn = [persist.tile([128, SQL], BF16, name=f"oTn{c}") for c in range(DC)]
            out_sb = [persist.tile([128, SQL], F32, name=f"osb{c}") for c in range(DC)]

            # zero the unused halves/padding once (gpsimd engine)
            for h in range(H):
                off = 64 * (h % 2)
                nc.gpsimd.memset(qrz[h][64 - off:128 - off, :], 0.0)
            for kc in range(SKC):
                nc.gpsimd.memset(vp[kc][:, :, HD + 1:128], 0.0)

            # ---- weight + input loads -----------------------------------
            def wfull(dram, dt):
                ws = []
                for c in range(DC):
                    t = tmp.tile([128, D], dt, tag="wfull", bufs=20, name="wf")
                    nc.scalar.dma_start(out=t, in_=dram[c * 128:(c + 1) * 128, :])
                    ws.append(t)
                return ws

            wk = wfull(wk_d, F16)                      # scalar queue, first
            qT = [ph1.tile([128, SQL], F16, name=f"qT{c}") for c in range(DC)]
            kvT = [ph1.tile([128, SK], F16, name=f"kvT{c}") for c in range(DC)]
            for c in range(DC):
                nc.sync.dma_start(out=kvT[c], in_=kvT_d[c * 128:(c + 1) * 128, :])
                nc.scalar.dma_start(out=qT[c], in_=qT_d[c * 128:(c + 1) * 128, :])

            # ---- RMSNorm over features (partition dim) ------------------
            ss = pp.tile([128, SQL], F32, tag="pp")
            for c in range(DC):
                sq = tmp.tile([128, SQL], F16, tag="sq", bufs=3, name="sq")
                nc.vector.tensor_mul(out=sq, in0=qT[c], in1=qT[c])
                nc.tensor.matmul(ss, ones_sb, sq, start=(c == 0), stop=(c == DC - 1))
            lnv = tmp.tile([128, SQL], F32, tag="lnv", name="lnv")
            nc.scalar.activation(out=lnv, in_=ss, func=AF.Ln, bias=eps_t,
                                 scale=1.0 / D)
            rstd = tmp.tile([128, SQL], F32, tag="rstd", name="rstd")
            nc.scalar.activation(out=rstd, in_=lnv, func=AF.Exp, scale=-0.5)
            for c in range(DC):
                nc.vector.tensor_mul(out=qT[c], in0=qT[c], in1=rstd)

            # ---- projection + RoPE (m-outer, resident weights) ----------
            def proj_rope(wt, xs, xsl, b_sb, sin_dram, cos_dram, coff,
                          emit_rot):
                pk = [big.tile([128, 3 * SQL], F32, tag="big", name=f"pk{j}")
                      for j in range(2)]
                for m in range(DC):
                    j, mm = divmod(m, 3)
                    sl = pk[j][:, mm * SQL:(mm + 1) * SQL]
                    for c in range(DC):
                        nc.tensor.matmul(sl, wt[c][:, m * 128:(m + 1) * 128],
                                         xs[c][:, xsl],
                                         start=(c == 0), stop=(c == DC - 1))
                    sin_t = tmp.tile([128, SQL], F16, tag="tbl", bufs=4,
                                     name="sin_t")
                    cos_t = tmp.tile([128, SQL], F16, tag="tbl", bufs=4,
                                     name="cos_t")
                    nc.sync.dma_start(
                        out=sin_t,
                        in_=sin_dram[m * 128:(m + 1) * 128, coff:coff + SQL])
                    nc.sync.dma_start(
                        out=cos_t,
                        in_=cos_dram[m * 128:(m + 1) * 128, coff:coff + SQL])
                    # biased projection to SBUF, then a 32-block swapped copy
                    kp = tmp.tile([128, SQL], F32R, tag="kp", bufs=4, name="kp")
                    nc.vector.tensor_scalar_add(out=kp, in0=sl,
                                                scalar1=b_sb[:, m:m + 1])
                    t1 = tmp.tile([128, SQL], F32R, tag="sw", bufs=4, name="t1")
                    for base in (0, 64):
                        nc.scalar.dma_start(out=t1[base:base + 32, :],
                                            in_=kp[base + 32:base + 64, :])
                        nc.scalar.dma_start(out=t1[base + 32:base + 64, :],
                                            in_=kp[base:base + 32, :])
                    nc.vector.tensor_mul(out=t1, in0=t1, in1=sin_t)
                    emit_rot(m, kp, t1, cos_t)

            def rot_k(hs):
                def emit(m, kp, t1, cos_t):
                    dst = krot[m][:, hs]
                    nc.vector.tensor_mul(out=dst, in0=kp, in1=cos_t)
                    nc.vector.tensor_add(out=dst, in0=dst, in1=t1)
                return emit

            def rot_q(m, kp, t1, cos_t):
                for h, pr in ((2 * m, slice(0, 64)), (2 * m + 1, slice(64, 128))):
                    dst = qrz[h][pr, :]
                    nc.vector.tensor_mul(out=dst, in0=kp[pr, :],
                                         in1=cos_t[pr, :])
                    nc.vector.tensor_add(out=dst, in0=dst, in1=t1[pr, :])

            proj_rope(wk, kvT, slice(0, SQL), bk_sb, sink_d, cosk_d,
                      0, rot_k(slice(0, SQL)))
            wq = wfull(wq_d, F16)
            proj_rope(wk, kvT, slice(SQL, SK), bk_sb, sink_d, cosk_d,
                      SQL, rot_k(slice(SQL, SK)))
            wv = wfull(wv_d, F16)
            proj_rope(wq, qT, slice(0, SQL), bq_sb, sinq_d, cosq_d,
                      0, rot_q)

            # ---- V projection (row-major, ones column at 64) ------------
            for kc in range(SKC):
                ksl = slice(kc * 128, (kc + 1) * 128)
                pv = big.tile([128, 3 * SQL], F32, tag="big", name="pv")
                for c in range(DC):
                    nc.tensor.matmul(pv[:, 0:512], kvT[c][:, ksl],
                                     wv[c][:, 0:512],
                                     start=(c == 0), stop=(c == DC - 1))
                    nc.tensor.matmul(pv[:, 512:768], kvT[c][:, ksl],
                                     wv[c][:, 512:768],
                                     start=(c == 0), stop=(c == DC - 1))
                nc.vector.tensor_copy(
                    out=vp[kc][:, 0:8, 0:HD],
                    in_=pv[:, 0:512].rearrange("p (h d) -> p h d", h=8))
                nc.vector.tensor_copy(
                    out=vp[kc][:, 8:12, 0:HD],
                    in_=pv[:, 512:768].rearrange("p (h d) -> p h d", h=4))
                nc.gpsimd.dma_start(out=vp[kc][:, :, HD], in_=onesc_sb)

            # ---- attention + pipelined per-head normalization -----------
            import concourse.bass as bass_mod
            GROUPS = [(0, 3), (3, 6), (6, 8)]
            for h in range(H):
                mh, off = h // 2, 64 * (h % 2)
                po = pp.tile([128, SQL], F32, tag="pp", name="po")
                for k0, k1 in GROUPS:
                    w = (k1 - k0) * SQL
                    sc = big.tile([128, 3 * SQL], F32, tag="big", name="sc")
                    for i, kc in enumerate(range(k0, k1)):
                        nc.tensor.matmul(
                            sc[:, i * SQL:(i + 1) * SQL],
                            krot[mh][:, kc * 128:(kc + 1) * 128],
                            qrz[h], start=True, stop=True)
                    ex = tmp.tile([128, 3 * SQL], BF16, tag="ex", bufs=3,
                                  name="ex")
                    nc.scalar.activation(out=ex[:, :w], in_=sc[:, :w],
                                         func=AF.Exp, scale=1.0 / 8.0)
                    for i, kc in enumerate(range(k0, k1)):
                        nc.tensor.matmul(po, vp[kc][:, h, :],
                                         ex[:, i * SQL:(i + 1) * SQL],
                                         start=(kc == 0), stop=(kc == SKC - 1))
                # denominator: reciprocal straight off PSUM, DRAM broadcast
                rden = tmp.tile([1, SQL], F32, tag="rden", bufs=3, name="rden")
                nc.vector.reciprocal(out=rden, in_=po[64:65, :])
                nc.sync.dma_start(out=rden_d[h:h + 1, :], in_=rden)
                rb = tmp.tile([64, SQL], F32, tag="rb", bufs=3, name="rb")
                row = rden_d[h, :]
                bsrc = bass_mod.AP(tensor=row.tensor, offset=row.offset,
                                   ap=[[0, 64], *row.ap])
                nc.gpsimd.dma_start(out=rb, in_=bsrc)
                nc.vector.tensor_mul(out=oTn[mh][off:off + 64, :],
                                     in0=po[0:64, :], in1=rb)

            # ---- output projection tail (c-outer, 2 psum accumulators) --
            wo = wfull(wo_d, BF16)
            po6 = [big.tile([128, 3 * SQL], F32, tag="big", name=f"po6_{j}")
                   for j in range(2)]
            for c in range(DC):
                for j in range(2):
                    for mm in range(3):
                        m = 3 * j + mm
                        nc.tensor.matmul(po6[j][:, mm * SQL:(mm + 1) * SQL],
                                         wo[c][:, m * 128:(m + 1) * 128],
                                         oTn[c], start=(c == 0),
                                         stop=(c == DC - 1))
            for m in range(DC):
                j, mm = divmod(m, 3)
                nc.vector.tensor_scalar_add(
                    out=out_sb[m], in0=po6[j][:, mm * SQL:(mm + 1) * SQL],
                    scalar1=bo_sb[:, m:m + 1])
                nc.sync.dma_start(out=out_d[m * 128:(m + 1) * 128, :],
                                  in_=out_sb[m])

    return nc


# ---------------------------------------------------------------------------
# host wrapper
# ---------------------------------------------------------------------------

def _rope_tables(pos, freqs):
    """pos [S,2] int, freqs [2,H,32] -> signed-sin and cos tables [D,S] fp16,
    rows in de-interleaved head-dim order (pair j at rows h*64+j / h*64+32+j,
    first-half sin rows negated so rot = x*cos + swap(x)*sin)."""
    ang = np.einsum('sd,dhj->hjs', pos.astype(np.float64),
                    freqs.astype(np.float64))          # [H,32,S]
    s, c = np.sin(ang), np.cos(ang)
    sin_full = np.concatenate([-s, s], axis=1).reshape(D, -1)
    cos_full = np.concatenate([c, c], axis=1).reshape(D, -1)
    return sin_full.astype(np.float16), cos_full.astype(np.float16)


def kernel(q, kv, posq, posk, w_norm, w_q, b_q, w_kv, b_kv, w_out, b_out, freqs):
    _apply_patches()
    from concourse.bass_utils import run_bass_kernel_spmd
    import ml_dtypes

    bf16 = ml_dtypes.bfloat16

    q = np.asarray(q, np.float32)
    kv = np.asarray(kv, np.float32)
    posq_np = np.asarray(posq)
    posk_np = np.asarray(posk)
    w_norm = np.asarray(w_norm, np.float32)
    w_q = np.asarray(w_q, np.float32)
    b_q = np.asarray(b_q, np.float32)
    w_kv = np.asarray(w_kv, np.float32)
    b_kv = np.asarray(b_kv, np.float32)
    w_out = np.asarray(w_out, np.float32)
    b_out = np.asarray(b_out, np.float32)
    freqs = np.asarray(freqs, np.float32)

    # de-interleave head dims: new j<32 -> old 2j (even), j>=32 -> old 2(j-32)+1
    perm = np.empty(D, np.int64)
    for h in range(H):
        for j in range(HD):
            perm[h * HD + j] = h * HD + (2 * j if j < 32 else 2 * (j - 32) + 1)

    wqT = np.ascontiguousarray((w_q[perm, :] * w_norm[None, :]).T).astype(np.float16)
    wkT = np.ascontiguousarray(w_kv[:D][perm, :].T).astype(np.float16)
    wvT = np.ascontiguousarray(w_kv[D:].T).astype(np.float16)
    woT = np.ascontiguousarray(w_out.T).astype(bf16)
    bqR = np.ascontiguousarray(b_q[perm].reshape(DC, 128).T)
    bkR = np.ascontiguousarray(b_kv[:D][perm].reshape(DC, 128).T)
    bo_eff = b_out + w_out @ b_kv[D:]          # fold V bias (softmax sums to 1)
    boR = np.ascontiguousarray(bo_eff.reshape(DC, 128).T)

    ones128 = np.ones((128, 128), np.float16)
    onescol = np.ones((128, H), bf16)

    if "nc" not in _cache:
        _cache["nc"] = _build_nc()
    nc = _cache["nc"]

    in_maps = []
    for core in range(NCORES):
        b, half = core // 2, core % 2
        qs = slice(half * SQL, (half + 1) * SQL)
        sinq, cosq = _rope_tables(posq_np[b], freqs)
        sink, cosk = _rope_tables(posk_np[b], freqs)
        in_maps.append({
            "qT": np.ascontiguousarray(q[b, qs, :].T).astype(np.float16),
            "kvT": np.ascontiguousarray(kv[b].T).astype(np.float16),
            "sinq": np.ascontiguousarray(sinq[:, qs]),
            "cosq": np.ascontiguousarray(cosq[:, qs]),
            "sink": np.ascontiguousarray(sink),
            "cosk": np.ascontiguousarray(cosk),
            "wqT": wqT, "wkT": wkT, "wvT": wvT, "woT": woT,
            "bqR": bqR, "bkR": bkR, "boR": boR,
            "ones128": ones128, "onescol": onescol,
        })

    res = run_bass_kernel_spmd(nc, in_maps, core_ids=list(range(NCORES)))
    kernel._last_result = res

    out = np.empty((B, SQ, D), np.float32)
    for core in range(NCORES):
        b, half = core // 2, core % 2
        out[b, half * SQL:(half + 1) * SQL, :] = res.results[core]["outT"].T
    return out
